# revision 5
# baseline (speedup 1.0000x reference)
"""Bahdanau attention on 8 Trainium2 NeuronCores.

ctx, a = attention(H, s, mask, Wh, Ws, bs, v):
  hs  = H @ Wh.T + (s @ Ws.T + bs)[:, None, :]
  e   = tanh(hs) @ v, masked
  a   = softmax(e, axis=T)
  ctx = einsum('bte,bt->be', H, a)

Sharding: data-parallel over batch B=64 across 8 cores (8 batches each);
the small weights Wh/Ws/bs/v are replicated.

Device-side layout (per core):
  - H is uploaded pre-transposed as HT [8, E, T] so the hs matmul can
    contract over E on SBUF partitions (fp32r, full PE rate).
  - hs is produced as hs.T tiles [A_chunk=128, T_chunk=512] in PSUM;
    tanh + (Ws@s+bs) bias are fused into one ACT activation per tile.
  - e = v.T @ tanh(hs.T) via PE (contracts A on partitions).
  - softmax uses a fixed shift c = sum|v| (a rigorous upper bound on
    max(e) since |tanh|<=1), so no second pass over e is needed.
  - ctx = sum_t u_t * HT[:, t] is a free-axis fused multiply-reduce on
    the vector engine (contraction over T without re-laying-out H), with
    u broadcast across partitions by GPSIMD.
"""

import sys

if "/opt/trn_rl_repo" not in sys.path:
    sys.path.insert(0, "/opt/trn_rl_repo")

from contextlib import ExitStack

import numpy as np

N_CORES = 8
B, T, E, A, D = 64, 2048, 1024, 512, 1024
BL = B // N_CORES  # local batches per core
TF = 512  # T tile (free dim of the hs matmul; fp32 moving-operand max)
TC = T // TF  # 4 T chunks
EC = E // 128  # 8 E chunks (contraction for hs)
AC = A // 128  # 4 A chunks (partition dim of hs.T / contraction for e)
DC = D // 128  # 8 D chunks (contraction for Ws @ s)

_BUILT = {}


def _build(n_batches=BL):
    import concourse.bass as bass  # noqa: F401
    import concourse.tile as tile
    from concourse import bacc, mybir

    f32 = mybir.dt.float32
    f32r = mybir.dt.float32r
    Act = mybir.ActivationFunctionType
    Alu = mybir.AluOpType
    Axis = mybir.AxisListType

    nc = bacc.Bacc("TRN2", target_bir_lowering=False, debug=False)

    ht = nc.dram_tensor("ht", [n_batches, E, T], f32r, kind="ExternalInput").ap()
    st = nc.dram_tensor("st", [D, n_batches], f32r, kind="ExternalInput").ap()
    mask01 = nc.dram_tensor("mask01", [n_batches, T], f32, kind="ExternalInput").ap()
    wht = nc.dram_tensor("wht", [E, A], f32r, kind="ExternalInput").ap()
    wst = nc.dram_tensor("wst", [D, A], f32r, kind="ExternalInput").ap()
    bs_d = nc.dram_tensor("bs_s", [128, AC], f32, kind="ExternalInput").ap()
    v_d = nc.dram_tensor("v_s", [128, AC], f32r, kind="ExternalInput").ap()
    negc_d = nc.dram_tensor("negc", [1, 1], f32, kind="ExternalInput").ap()
    ctx_out = nc.dram_tensor("ctx_out", [n_batches, E], f32, kind="ExternalOutput").ap()
    a_out = nc.dram_tensor("a_out", [n_batches, T], f32, kind="ExternalOutput").ap()

    with tile.TileContext(nc) as tc, ExitStack() as ctx:
        weights = ctx.enter_context(tc.tile_pool(name="weights", bufs=1))
        htp = ctx.enter_context(tc.tile_pool(name="htp", bufs=3))
        tanhp = ctx.enter_context(tc.tile_pool(name="tanhp", bufs=2))
        smalls = ctx.enter_context(tc.tile_pool(name="smalls", bufs=1))
        rowp = ctx.enter_context(tc.tile_pool(name="rowp", bufs=2))
        scr = ctx.enter_context(tc.tile_pool(name="scr", bufs=2))
        psum_hs = ctx.enter_context(tc.tile_pool(name="psum_hs", bufs=5, space="PSUM"))
        psum_e = ctx.enter_context(tc.tile_pool(name="psum_e", bufs=2, space="PSUM"))

        # ---- setup: load replicated weights ----
        wht_s = weights.tile([128, EC, A], f32r)
        nc.sync.dma_start(wht_s[:], wht.rearrange("(k p) a -> p k a", p=128))
        v_s = smalls.tile([128, AC], f32r)
        nc.sync.dma_start(v_s[:], v_d[:])
        bs_s = smalls.tile([128, AC], f32)
        nc.sync.dma_start(bs_s[:], bs_d[:])
        negc_s = smalls.tile([1, 1], f32)
        nc.sync.dma_start(negc_s[:], negc_d[:])
        sT_s = smalls.tile([128, DC, n_batches], f32r)
        nc.sync.dma_start(sT_s[:], st.rearrange("(k p) b -> p k b", p=128))

        # sb[a, b] = (Ws @ s_b + bs)[a], laid out [128, AC, n_batches].
        # wst borrows an htp slot (same size as an ht tile) so it doesn't
        # cost SBUF after setup.
        wst_s = htp.tile([128, EC, TF], f32r, tag="ht")
        nc.sync.dma_start(wst_s[:], wst.rearrange("(k p) a -> p k a", p=128))
        sb_s = smalls.tile([128, AC, n_batches], f32)
        for am in range(AC):
            pe_sb = psum_e.tile([128, n_batches], f32, tag="e")
            for dk in range(DC):
                nc.tensor.matmul(
                    pe_sb[:],
                    wst_s[:, dk, am * 128 : (am + 1) * 128],
                    sT_s[:, dk, :],
                    start=(dk == 0),
                    stop=(dk == DC - 1),
                )
            nc.vector.tensor_scalar_add(sb_s[:, am, :], pe_sb[:], bs_s[:, am : am + 1])

        # ---- main loop over local batches ----
        for b in range(n_batches):
            mask_sb = rowp.tile([1, T], f32, tag="mask")
            nc.sync.dma_start(mask_sb[:], mask01[b : b + 1, :])
            u_all = rowp.tile([1, T], f32, tag="u")
            ctx_parts = rowp.tile([128, EC, TC], f32, tag="cp")

            for c in range(TC):
                ht_t = htp.tile([128, EC, TF], f32r, tag="ht")
                nc.sync.dma_start(
                    ht_t[:],
                    ht[b].rearrange("(k p) t -> p k t", p=128)[
                        :, :, c * TF : (c + 1) * TF
                    ],
                )

                # hs.T tiles: [A chunk 128, TF] accumulated over E chunks
                tanh_t = tanhp.tile([128, AC, TF], f32r, tag="tanh")
                for am in range(AC):
                    ph = psum_hs.tile([128, TF], f32, tag="hs")
                    for ek in range(EC):
                        nc.tensor.matmul(
                            ph[:],
                            wht_s[:, ek, am * 128 : (am + 1) * 128],
                            ht_t[:, ek, :],
                            start=(ek == 0),
                            stop=(ek == EC - 1),
                        )
                    # tanh(hs + (Ws s + bs)) with the bias fused in
                    nc.scalar.activation(
                        tanh_t[:, am, :],
                        ph[:],
                        Act.Tanh,
                        bias=sb_s[:, am, b : b + 1],
                    )

                # e = v.T @ tanh(hs.T): contract A on partitions
                pe = psum_e.tile([1, TF], f32, tag="e")
                for am in range(AC):
                    nc.tensor.matmul(
                        pe[:],
                        v_s[:, am : am + 1],
                        tanh_t[:, am, :],
                        start=(am == 0),
                        stop=(am == AC - 1),
                    )

                # u = exp(e - c); c = sum|v| >= max(e) always
                u_c = scr.tile([1, TF], f32, tag="u_c")
                nc.scalar.activation(u_c[:], pe[:], Act.Exp, bias=negc_s[0:1, :])

                # apply the mask (tensor_tensor_reduce is broken on this
                # HW build, so mask-mult and Z-reduce are separate ops)
                nc.vector.tensor_mul(
                    u_all[:, c * TF : (c + 1) * TF],
                    u_c[:],
                    mask_sb[:, c * TF : (c + 1) * TF],
                )

                # broadcast u across partitions for the ctx reduction
                u_rep = scr.tile([128, TF], f32, tag="u_rep")
                nc.gpsimd.partition_broadcast(
                    u_rep[:], u_all[:, c * TF : (c + 1) * TF]
                )

                # ctx partials: sum_t HT[e, t] * u[t] per E chunk.
                # multiply on GPSIMD (otherwise idle), free-axis reduce on DVE
                for ek in range(EC):
                    tout = scr.tile([128, TF], f32, tag="tout")
                    nc.gpsimd.tensor_mul(
                        tout[:], ht_t[:, ek, :].bitcast(f32), u_rep[:]
                    )
                    nc.vector.tensor_reduce(
                        ctx_parts[:, ek, c : c + 1],
                        tout[:],
                        axis=Axis.X,
                        op=Alu.add,
                    )

            # ---- finalize batch ----
            z_b = rowp.tile([1, 1], f32, tag="zb")
            nc.vector.tensor_reduce(z_b[:], u_all[:], axis=Axis.X, op=Alu.add)
            rz = rowp.tile([1, 1], f32, tag="rz")
            nc.vector.reciprocal(rz[:], z_b[:])

            a_sb = rowp.tile([1, T], f32, tag="a")
            nc.scalar.activation(a_sb[:], u_all[:], Act.Copy, scale=rz[0:1, :])
            nc.sync.dma_start(a_out[b : b + 1, :], a_sb[:])

            ctx_u = rowp.tile([128, EC], f32, tag="cu")
            nc.vector.tensor_reduce(ctx_u[:], ctx_parts[:], axis=Axis.X, op=Alu.add)
            rz_rep = scr.tile([128, 1], f32, tag="rz_rep")
            nc.gpsimd.partition_broadcast(rz_rep[:], rz[:])
            ctx_f = rowp.tile([128, EC], f32, tag="cf")
            nc.vector.tensor_scalar_mul(ctx_f[:], ctx_u[:], rz_rep[:])
            nc.sync.dma_start(ctx_out[b].rearrange("(k p) -> p k", p=128), ctx_f[:])

    nc.compile()
    return nc


def _get_built(n_batches=BL):
    if n_batches not in _BUILT:
        _BUILT[n_batches] = _build(n_batches)
    return _BUILT[n_batches]


def make_in_maps(H, s, mask, Wh, Ws, bs, v, n_cores=N_CORES):
    """Host-side sharding/prep. Returns the per-core input maps."""
    H = np.asarray(H, dtype=np.float32)
    s = np.asarray(s, dtype=np.float32)
    mask = np.asarray(mask)
    Wh = np.asarray(Wh, dtype=np.float32)
    Ws = np.asarray(Ws, dtype=np.float32)
    bs = np.asarray(bs, dtype=np.float32)
    v = np.asarray(v, dtype=np.float32)

    whT = np.ascontiguousarray(Wh.T)  # [E, A]
    wsT = np.ascontiguousarray(Ws.T)  # [D, A]
    bs_s = np.ascontiguousarray(bs.reshape(AC, 128).T)  # [128, AC]
    v_s = np.ascontiguousarray(v.reshape(AC, 128).T)  # [128, AC]
    negc = np.array([[-np.sum(np.abs(v))]], dtype=np.float32)

    bl = H.shape[0] // n_cores
    in_maps = []
    for i in range(n_cores):
        sl = slice(i * bl, (i + 1) * bl)
        in_maps.append(
            {
                "ht": np.ascontiguousarray(H[sl].transpose(0, 2, 1)),
                "st": np.ascontiguousarray(s[sl].T),
                "mask01": mask[sl].astype(np.float32),
                "wht": whT,
                "wst": wsT,
                "bs_s": bs_s,
                "v_s": v_s,
                "negc": negc,
            }
        )
    return in_maps


def kernel(H, s, mask, Wh, Ws, bs, v):
    from concourse.bass_utils import run_bass_kernel_spmd

    nc = _get_built(BL)
    in_maps = make_in_maps(H, s, mask, Wh, Ws, bs, v)
    res = run_bass_kernel_spmd(nc, in_maps, core_ids=list(range(N_CORES)))
    ctx = np.concatenate([res.results[i]["ctx_out"] for i in range(N_CORES)], axis=0)
    a = np.concatenate([res.results[i]["a_out"] for i in range(N_CORES)], axis=0)
    return ctx, a


# revision 7
# speedup vs baseline: 2140.2105x; 2140.2105x over previous
"""Bahdanau attention on 8 Trainium2 NeuronCores.

ctx, a = attention(H, s, mask, Wh, Ws, bs, v):
  hs  = H @ Wh.T + (s @ Ws.T + bs)[:, None, :]
  e   = tanh(hs) @ v, masked
  a   = softmax(e, axis=T)
  ctx = einsum('bte,bt->be', H, a)

Sharding: data-parallel over batch B=64 across 8 cores (8 batches each);
the small weights Wh/Ws/bs/v are replicated.

Device-side layout (per core):
  - H is uploaded pre-transposed as HT [8, E, T] so the hs matmul can
    contract over E on SBUF partitions (fp32r, full PE rate).
  - hs is produced as hs.T tiles [A_chunk=128, T_chunk=512] in PSUM;
    tanh + (Ws@s+bs) bias are fused into one ACT activation per tile.
  - e = v.T @ tanh(hs.T) via PE (contracts A on partitions).
  - softmax uses a fixed shift c = sum|v| (a rigorous upper bound on
    max(e) since |tanh|<=1), so no second pass over e is needed.
  - ctx = sum_t u_t * HT[:, t] is a free-axis fused multiply-reduce on
    the vector engine (contraction over T without re-laying-out H), with
    u broadcast across partitions by GPSIMD.
"""

import sys

if "/opt/trn_rl_repo" not in sys.path:
    sys.path.insert(0, "/opt/trn_rl_repo")

from contextlib import ExitStack

import numpy as np

N_CORES = 8
B, T, E, A, D = 64, 2048, 1024, 512, 1024
BL = B // N_CORES  # local batches per core
TF = 512  # T tile (free dim of the hs matmul; fp32 moving-operand max)
TC = T // TF  # 4 T chunks
EC = E // 128  # 8 E chunks (contraction for hs)
AC = A // 128  # 4 A chunks (partition dim of hs.T / contraction for e)
DC = D // 128  # 8 D chunks (contraction for Ws @ s)

_BUILT = {}


def _build(n_batches=BL, variant="full", repeat=1):
    import concourse.bass as bass  # noqa: F401
    import concourse.tile as tile
    from concourse import bacc, mybir

    f32 = mybir.dt.float32
    f32r = mybir.dt.float32r
    Act = mybir.ActivationFunctionType
    Alu = mybir.AluOpType
    Axis = mybir.AxisListType

    nc = bacc.Bacc("TRN2", target_bir_lowering=False, debug=False)

    ht = nc.dram_tensor("ht", [n_batches, E, T], f32r, kind="ExternalInput").ap()
    st = nc.dram_tensor("st", [D, n_batches], f32r, kind="ExternalInput").ap()
    mask01 = nc.dram_tensor("mask01", [n_batches, T], f32, kind="ExternalInput").ap()
    wht = nc.dram_tensor("wht", [E, A], f32r, kind="ExternalInput").ap()
    wst = nc.dram_tensor("wst", [D, A], f32r, kind="ExternalInput").ap()
    bs_d = nc.dram_tensor("bs_s", [128, AC], f32, kind="ExternalInput").ap()
    v_d = nc.dram_tensor("v_s", [128, AC], f32r, kind="ExternalInput").ap()
    negc_d = nc.dram_tensor("negc", [1, 1], f32, kind="ExternalInput").ap()
    ctx_out = nc.dram_tensor("ctx_out", [n_batches, E], f32, kind="ExternalOutput").ap()
    a_out = nc.dram_tensor("a_out", [n_batches, T], f32, kind="ExternalOutput").ap()

    with tile.TileContext(nc) as tc, ExitStack() as ctx:
        weights = ctx.enter_context(tc.tile_pool(name="weights", bufs=1))
        htp = ctx.enter_context(tc.tile_pool(name="htp", bufs=3))
        tanhp = ctx.enter_context(tc.tile_pool(name="tanhp", bufs=2))
        smalls = ctx.enter_context(tc.tile_pool(name="smalls", bufs=1))
        rowp = ctx.enter_context(tc.tile_pool(name="rowp", bufs=2))
        scr = ctx.enter_context(tc.tile_pool(name="scr", bufs=2))
        psum_hs = ctx.enter_context(tc.tile_pool(name="psum_hs", bufs=5, space="PSUM"))
        psum_e = ctx.enter_context(tc.tile_pool(name="psum_e", bufs=2, space="PSUM"))

        # ---- setup: load replicated weights ----
        wht_s = weights.tile([128, EC, A], f32r)
        nc.sync.dma_start(wht_s[:], wht.rearrange("(k p) a -> p k a", p=128))
        v_s = smalls.tile([128, AC], f32r)
        nc.sync.dma_start(v_s[:], v_d[:])
        bs_s = smalls.tile([128, AC], f32)
        nc.sync.dma_start(bs_s[:], bs_d[:])
        negc_s = smalls.tile([1, 1], f32)
        nc.sync.dma_start(negc_s[:], negc_d[:])
        sT_s = smalls.tile([128, DC, n_batches], f32r)
        nc.sync.dma_start(sT_s[:], st.rearrange("(k p) b -> p k b", p=128))

        # sb[a, b] = (Ws @ s_b + bs)[a], laid out [128, AC, n_batches].
        # wst borrows an htp slot (same size as an ht tile) so it doesn't
        # cost SBUF after setup.
        wst_s = htp.tile([128, EC, TF], f32r, tag="ht")
        nc.sync.dma_start(wst_s[:], wst.rearrange("(k p) a -> p k a", p=128))
        sb_s = smalls.tile([128, AC, n_batches], f32)
        for am in range(AC):
            pe_sb = psum_e.tile([128, n_batches], f32, tag="e")
            for dk in range(DC):
                nc.tensor.matmul(
                    pe_sb[:],
                    wst_s[:, dk, am * 128 : (am + 1) * 128],
                    sT_s[:, dk, :],
                    start=(dk == 0),
                    stop=(dk == DC - 1),
                )
            nc.vector.tensor_scalar_add(sb_s[:, am, :], pe_sb[:], bs_s[:, am : am + 1])

        # ---- main loop over local batches ----
        # (repeat > 1 re-runs the whole loop for slope-based timing:
        #  per-call constants cancel in t(R2) - t(R1))
        for b in [bb for _ in range(repeat) for bb in range(n_batches)]:
            mask_sb = rowp.tile([1, T], f32, tag="mask")
            nc.sync.dma_start(mask_sb[:], mask01[b : b + 1, :])
            u_all = rowp.tile([1, T], f32, tag="u")
            ctx_parts = rowp.tile([128, EC, TC], f32, tag="cp")
            if variant in ("noctx", "matonly", "noexp"):
                nc.vector.memset(ctx_parts[:], 0.0)
            if variant in ("matonly", "noexp"):
                nc.vector.memset(u_all[:], 1.0)

            for c in range(TC):
                ht_t = htp.tile([128, EC, TF], f32r, tag="ht")
                nc.sync.dma_start(
                    ht_t[:],
                    ht[b].rearrange("(k p) t -> p k t", p=128)[
                        :, :, c * TF : (c + 1) * TF
                    ],
                )

                # hs.T tiles: [A chunk 128, TF] accumulated over E chunks
                tanh_t = tanhp.tile([128, AC, TF], f32r, tag="tanh")
                for am in range(AC):
                    ph = psum_hs.tile([128, TF], f32, tag="hs")
                    for ek in range(EC):
                        nc.tensor.matmul(
                            ph[:],
                            wht_s[:, ek, am * 128 : (am + 1) * 128],
                            ht_t[:, ek, :],
                            start=(ek == 0),
                            stop=(ek == EC - 1),
                        )
                    # tanh(hs + (Ws s + bs)) with the bias fused in
                    nc.scalar.activation(
                        tanh_t[:, am, :],
                        ph[:],
                        Act.Tanh,
                        bias=sb_s[:, am, b : b + 1],
                    )

                if variant == "matonly":
                    continue
                # e = v.T @ tanh(hs.T): contract A on partitions
                pe = psum_e.tile([1, TF], f32, tag="e")
                for am in range(AC):
                    nc.tensor.matmul(
                        pe[:],
                        v_s[:, am : am + 1],
                        tanh_t[:, am, :],
                        start=(am == 0),
                        stop=(am == AC - 1),
                    )

                # u = exp(e - c); c = sum|v| >= max(e) always
                u_c = scr.tile([1, TF], f32, tag="u_c")
                nc.scalar.activation(u_c[:], pe[:], Act.Exp, bias=negc_s[0:1, :])

                if variant == "noexp":
                    continue
                # apply the mask (tensor_tensor_reduce is broken on this
                # HW build, so mask-mult and Z-reduce are separate ops)
                nc.vector.tensor_mul(
                    u_all[:, c * TF : (c + 1) * TF],
                    u_c[:],
                    mask_sb[:, c * TF : (c + 1) * TF],
                )

                # broadcast u across partitions for the ctx reduction
                if variant in ("full", "dvemul"):
                    u_rep = scr.tile([128, TF], f32, tag="u_rep")
                    nc.gpsimd.partition_broadcast(
                        u_rep[:], u_all[:, c * TF : (c + 1) * TF]
                    )

                # ctx partials: sum_t HT[e, t] * u[t] per E chunk.
                # multiply on GPSIMD (otherwise idle), free-axis reduce on DVE
                if variant in ("full", "dvemul"):
                    for ek in range(EC):
                        tout = scr.tile([128, TF], f32, tag="tout")
                        if variant == "dvemul":
                            nc.vector.tensor_mul(
                                tout[:], ht_t[:, ek, :].bitcast(f32), u_rep[:]
                            )
                        else:
                            nc.gpsimd.tensor_mul(
                                tout[:], ht_t[:, ek, :].bitcast(f32), u_rep[:]
                            )
                        nc.vector.tensor_reduce(
                            ctx_parts[:, ek, c : c + 1],
                            tout[:],
                            axis=Axis.X,
                            op=Alu.add,
                        )
                elif variant == "redonly":
                    for ek in range(EC):
                        nc.vector.tensor_reduce(
                            ctx_parts[:, ek, c : c + 1],
                            ht_t[:, ek, :].bitcast(f32),
                            axis=Axis.X,
                            op=Alu.add,
                        )

            # ---- finalize batch ----
            z_b = rowp.tile([1, 1], f32, tag="zb")
            nc.vector.tensor_reduce(z_b[:], u_all[:], axis=Axis.X, op=Alu.add)
            rz = rowp.tile([1, 1], f32, tag="rz")
            nc.vector.reciprocal(rz[:], z_b[:])

            a_sb = rowp.tile([1, T], f32, tag="a")
            nc.scalar.activation(a_sb[:], u_all[:], Act.Copy, scale=rz[0:1, :])
            nc.sync.dma_start(a_out[b : b + 1, :], a_sb[:])

            ctx_u = rowp.tile([128, EC], f32, tag="cu")
            nc.vector.tensor_reduce(ctx_u[:], ctx_parts[:], axis=Axis.X, op=Alu.add)
            rz_rep = scr.tile([128, 1], f32, tag="rz_rep")
            nc.gpsimd.partition_broadcast(rz_rep[:], rz[:])
            ctx_f = rowp.tile([128, EC], f32, tag="cf")
            nc.vector.tensor_scalar_mul(ctx_f[:], ctx_u[:], rz_rep[:])
            nc.sync.dma_start(ctx_out[b].rearrange("(k p) -> p k", p=128), ctx_f[:])

    nc.compile()
    return nc


def _get_built(n_batches=BL):
    if n_batches not in _BUILT:
        _BUILT[n_batches] = _build(n_batches)
    return _BUILT[n_batches]


def make_in_maps(H, s, mask, Wh, Ws, bs, v, n_cores=N_CORES):
    """Host-side sharding/prep. Returns the per-core input maps."""
    H = np.asarray(H, dtype=np.float32)
    s = np.asarray(s, dtype=np.float32)
    mask = np.asarray(mask)
    Wh = np.asarray(Wh, dtype=np.float32)
    Ws = np.asarray(Ws, dtype=np.float32)
    bs = np.asarray(bs, dtype=np.float32)
    v = np.asarray(v, dtype=np.float32)

    whT = np.ascontiguousarray(Wh.T)  # [E, A]
    wsT = np.ascontiguousarray(Ws.T)  # [D, A]
    bs_s = np.ascontiguousarray(bs.reshape(AC, 128).T)  # [128, AC]
    v_s = np.ascontiguousarray(v.reshape(AC, 128).T)  # [128, AC]
    negc = np.array([[-np.sum(np.abs(v))]], dtype=np.float32)

    bl = H.shape[0] // n_cores
    in_maps = []
    for i in range(n_cores):
        sl = slice(i * bl, (i + 1) * bl)
        in_maps.append(
            {
                "ht": np.ascontiguousarray(H[sl].transpose(0, 2, 1)),
                "st": np.ascontiguousarray(s[sl].T),
                "mask01": mask[sl].astype(np.float32),
                "wht": whT,
                "wst": wsT,
                "bs_s": bs_s,
                "v_s": v_s,
                "negc": negc,
            }
        )
    return in_maps


def kernel(H, s, mask, Wh, Ws, bs, v):
    from concourse.bass_utils import run_bass_kernel_spmd

    nc = _get_built(BL)
    in_maps = make_in_maps(H, s, mask, Wh, Ws, bs, v)
    res = run_bass_kernel_spmd(nc, in_maps, core_ids=list(range(N_CORES)))
    ctx = np.concatenate([res.results[i]["ctx_out"] for i in range(N_CORES)], axis=0)
    a = np.concatenate([res.results[i]["a_out"] for i in range(N_CORES)], axis=0)
    return ctx, a
